# revision 2
# baseline (speedup 1.0000x reference)
"""Trainium2 kernel for the DepthTracker correlation pyramid.

Math: for each level l, frame t, track n, the reference bilinearly samples a
7x7 grid of points around coords[t,n] from fmaps_l (128 channels) and
correlates each sample with the 49 track features -> out (L,B,T,N,7,7,7,7).

Decomposition (host gathers + blends patches, device does the 10-GFLOP
correlation, 32 tracks per core, fully data parallel, no collectives):
  out[l,t,n,h,w,pq] = G[l,n,pq,(t,w,h)]
  G[l,n,pq,(t,w,h)] = sum_c trackT[c,(l,n,pq)] * feat[l,n,c,(t,w,h)]

Device layout (2-byte dtypes): the matmul computes G^T per track in
128-row chunks of the TUV=784 axis: out[tuv_chunk(128), pq(49)] =
patch_chunk[C=128,128]^T @ trackT[C,49]. Stationary weights are always
exactly 128 columns so the compiler's Fast Weight Load kicks in; chunks
step by 112 and rows 112:127 of each PSUM result are discarded (they
were computed from the next chunk's columns), so the SBUF/ HBM output
tile is a dense [112, NB*7*49] block with zero padding waste. Stores are
112-partition, 5488B-contiguous-per-partition DMAs: they avoid SDMA
engine 15 (which HW-measures ~17% slower than the others and is the
critical path of a 128-row-store layout) and carry no dead rows (the old
pq-on-partitions layout stored 128 rows with only 98 useful).

HBM traffic per core: 25.8 MB patch loads + 1.6 MB track + 9.8 MB stores
= 37.2 MB vs 40.1 MB for the old layout.

COMPUTE_DT='f32r' + OUT_DT='f32' is a slower, more precise fallback that
uses the old pq-on-partitions layout.
"""

import numpy as np

R = 3
K7 = 7
LEV = 4
B, T, C, N = 1, 16, 128, 256
H, W = 96, 128
NCORES = 8
NS = N // NCORES          # 32 tracks per core
UV = K7 * K7
TUV = T * UV              # 784
PQ = K7 * K7              # 49
CH = (512, 272)           # legacy path matmul free-dim chunks
NB = 8                    # tracks per load DMA / per store
CHUNK = 112               # G^T rows kept per matmul (7*112 = 784)
NCH = TUV // CHUNK        # 7 chunks per track
PAD = 16                  # weight-read overrun past the last track

COMPUTE_DT = 'f16'        # 'f32r' | 'f32' | 'f16' | 'bf16'
OUT_DT = 'f16'            # dtype of the device G output: 'f32' | 'f16'
TRACE = False             # set True to capture an NTFF profile (test.py only)
LAST_RESULT = {}          # phase timings + profile info for test.py

_BASS_CACHE = {}


def _np_compute_dtype():
    if COMPUTE_DT in ('f32r', 'f32'):
        return np.float32
    if COMPUTE_DT == 'f16':
        return np.float16
    import ml_dtypes
    return np.dtype(ml_dtypes.bfloat16)


def _build_bass():
    key = (COMPUTE_DT, OUT_DT)
    if key in _BASS_CACHE:
        return _BASS_CACHE[key]
    import concourse.bacc as bacc
    import concourse.mybir as mybir
    from concourse import tile

    cdt = {
        'f32r': mybir.dt.float32r,
        'f32': mybir.dt.float32,
        'f16': mybir.dt.float16,
        'bf16': mybir.dt.bfloat16,
    }[COMPUTE_DT]
    f32 = mybir.dt.float32
    odt = f32 if OUT_DT == 'f32' else mybir.dt.float16

    nc = bacc.Bacc("TRN2", target_bir_lowering=False, debug=False)
    gt = mybir.dt.size(cdt) == 2  # G^T dense-store layout
    # patches: c-major with each partition's data contiguous per level.
    # +PAD cols so the last track's final 128-col weight read stays in
    # bounds (rows computed from those columns are discarded).
    patches = nc.dram_tensor(
        "patches", (LEV, C, NS * TUV + (PAD if gt else 0)), cdt,
        kind="ExternalInput")
    trackT = nc.dram_tensor("trackT", (C, LEV * NS * PQ), cdt,
                            kind="ExternalInput")
    if gt:
        gout = nc.dram_tensor("gout", (LEV, NS // NB, CHUNK, NB * NCH * PQ),
                              odt, kind="ExternalOutput")
    else:
        gout = nc.dram_tensor("gout", (LEV, NS, PQ, TUV), odt,
                              kind="ExternalOutput")
    with tile.TileContext(nc) as tc:
        with (
            tc.tile_pool(name="track", bufs=1) as track_pool,
            tc.tile_pool(name="patch",
                         bufs=8 if gt else 3) as patch_pool,
            tc.tile_pool(name="out",
                         bufs=3 if mybir.dt.size(odt) == 2 else 2
                         ) as out_pool,
            tc.tile_pool(name="psum", bufs=8 if gt else 4,
                         space="PSUM") as psum_pool,
        ):
            tr = track_pool.tile([C, LEV * NS * PQ], cdt)
            for l in range(LEV):
                # per-level just-in-time track slice: keeps the first patch
                # load from queuing behind a full track preload
                ksl = l * NS * PQ
                nc.sync.dma_start(tr[:, ksl:ksl + NS * PQ],
                                  trackT[:, ksl:ksl + NS * PQ])
                for nb in range(NS // NB):
                    ext = PAD if gt else 0
                    pt = patch_pool.tile([C, NB * TUV + ext], cdt, tag="pt")
                    off = nb * NB * TUV
                    if l == 0 and nb == 0:
                        # split the first load so compute starts earlier
                        nc.sync.dma_start(
                            pt[:, :NB * TUV // 2],
                            patches[l, :, off:off + NB * TUV // 2])
                        nc.sync.dma_start(
                            pt[:, NB * TUV // 2:],
                            patches[l, :, off + NB * TUV // 2:
                                          off + NB * TUV + ext])
                    else:
                        nc.sync.dma_start(
                            pt[:], patches[l, :, off:off + NB * TUV + ext])
                    if gt:
                        ot = out_pool.tile([CHUNK, NB * NCH * PQ], odt,
                                           tag="ot")
                        for g in range(NB):
                            n = nb * NB + g
                            k = (l * NS + n) * PQ
                            ps = psum_pool.tile([128, 512], f32, tag="ps")
                            for j in range(NCH):
                                wofs = g * TUV + j * CHUNK
                                nc.tensor.matmul(
                                    ps[:, j * PQ:(j + 1) * PQ],
                                    pt[:, wofs:wofs + 128],
                                    tr[:, k:k + PQ],
                                    start=True, stop=True)
                            dst = ot[:, g * NCH * PQ:(g + 1) * NCH * PQ]
                            if g % 2 == 0:
                                nc.vector.tensor_copy(
                                    dst, ps[0:CHUNK, :NCH * PQ])
                            else:
                                nc.scalar.copy(dst, ps[0:CHUNK, :NCH * PQ])
                        nc.sync.dma_start(gout[l, nb], ot[:])
                    else:
                        ot = out_pool.tile([PQ, NB * TUV], odt, tag="ot")
                        for g in range(NB):
                            n = nb * NB + g
                            k = (l * NS + n) * PQ
                            ps = psum_pool.tile([128, TUV], f32, tag="ps")
                            o = 0
                            for w_ch in CH:
                                nc.tensor.matmul(
                                    ps[0:PQ, o:o + w_ch],
                                    tr[:, k:k + PQ],
                                    pt[:, g * TUV + o:g * TUV + o + w_ch],
                                    start=True, stop=True)
                                o += w_ch
                            dst = ot[0:PQ, g * TUV:(g + 1) * TUV]
                            if g % 2 == 0:
                                nc.vector.tensor_copy(dst, ps[0:PQ, :])
                            else:
                                nc.scalar.copy(dst, ps[0:PQ, :])
                        nc.sync.dma_start(
                            gout[l, nb * NB:(nb + 1) * NB].rearrange(
                                "g p v -> p g v"),
                            ot[:].rearrange("p (g v) -> p g v", g=NB))
    nc.compile()
    _BASS_CACHE[key] = nc
    return nc


def _blend_mats(xy, dim):
    """xy: (T,N) fp32 coords at this level's scale. Returns (origin (T,N)
    int32, S (T,N,7,8) fp32) with reference clamping semantics folded in."""
    d = np.arange(-R, R + 1, dtype=np.float32)
    q = xy[..., None] + d
    qc = np.clip(q, 0.0, dim - 1.0)
    x0 = np.floor(qc)
    w = (qc - x0).astype(np.float32)
    x0i = x0.astype(np.int32)
    x1i = np.minimum(x0i + 1, dim - 1)
    org = np.clip(np.floor(xy).astype(np.int32) - R, 0, dim - 8)
    v0 = x0i - org[..., None]
    v1 = x1i - org[..., None]
    eye = np.eye(8, dtype=np.float32)
    S = eye[v0] * (1.0 - w)[..., None] + eye[v1] * w[..., None]
    return org, S


def kernel(fmaps0, fmaps1, fmaps2, fmaps3, track0, track1, track2, track3,
           coords):
    import time as _time
    _t0 = _time.time()
    fmaps = [fmaps0, fmaps1, fmaps2, fmaps3]
    tracks = [track0, track1, track2, track3]
    cdt_np = _np_compute_dtype()
    gt = cdt_np().itemsize == 2
    coords2 = np.asarray(coords, np.float32)[0]        # (T,N,2)

    # ---- host: blend matrices + patch gather --------------------------------
    patches_all = np.empty((LEV, C, N, T, K7, K7), cdt_np)
    for l in range(LEV):
        Hl, Wl = H >> l, W >> l
        sc = np.float32(2.0 ** l)
        x = (coords2[..., 0] / sc).astype(np.float32)
        y = (coords2[..., 1] / sc).astype(np.float32)
        cx, Sx = _blend_mats(x, Wl)
        cy, Sy = _blend_mats(y, Hl)
        fm = np.asarray(fmaps[l], np.float32)[0]       # (T,C,Hl,Wl)
        iy = cy[..., None] + np.arange(8)              # (T,N,8)
        ix = cx[..., None] + np.arange(8)
        t_idx = np.arange(T)[:, None, None, None]
        # fancy indexing -> (T,N,8,8,C) over (u=y-row, v=x-col)
        p = fm[t_idx, :, iy[:, :, :, None], ix[:, :, None, :]]
        # x-blend: (T,N,1,7,8) @ (T,N,8,8,C) -> (T,N,8,7,C)  [u, h]
        px = np.matmul(Sx[:, :, None, :, :], p)
        # y-blend: (T,N,7,8) @ (T,N,8,7*C) -> (T,N,7,7,C)    [w, h]
        py = np.matmul(Sy, px.reshape(T, N, 8, K7 * C))
        py = py.reshape(T, N, K7, K7, C)
        patches_all[l] = py.transpose(4, 1, 0, 2, 3)   # (C,N,T,7,7)

    trackT_all = np.empty((C, LEV, N, PQ), cdt_np)
    for l in range(LEV):
        # track_l: (1,49,N,C) -> (C, N, PQ)
        trackT_all[:, l] = np.asarray(tracks[l], np.float32)[0].transpose(2, 1, 0)

    # ---- device: G = track^T @ patches, 32 tracks per core ------------------
    nc = _build_bass()
    from concourse import bass_utils
    pad = PAD if gt else 0
    in_maps = []
    for kc in range(NCORES):
        sl = slice(kc * NS, (kc + 1) * NS)
        pc = np.zeros((LEV, C, NS * TUV + pad), cdt_np)
        pc[:, :, :NS * TUV] = patches_all[:, :, sl].reshape(LEV, C, NS * TUV)
        in_maps.append({
            "patches": pc,
            "trackT": np.ascontiguousarray(
                trackT_all[:, :, sl].reshape(C, LEV * NS * PQ)),
        })
    _t1 = _time.time()
    res = bass_utils.run_bass_kernel_spmd(
        nc, in_maps, core_ids=list(range(NCORES)), trace=TRACE)
    _t2 = _time.time()
    LAST_RESULT.update(
        host_pre_s=_t1 - _t0, spmd_s=_t2 - _t1,
        exec_time_ns=res.exec_time_ns, profile_json=res.profile_json)
    if gt:
        # per core: gout (LEV, NS//NB, 112, NB*7*49): row p of chunk j of
        # track g is G^T[l, n=nb*NB+g, tuv=j*112+p, pq].
        GT = np.empty((LEV, NCORES, NS, TUV, PQ), np.float32)
        for kc, r in enumerate(res.results):
            g = r["gout"].reshape(LEV, NS // NB, CHUNK, NB, NCH, PQ)
            GT[:, kc] = g.transpose(0, 1, 3, 4, 2, 5).reshape(
                LEV, NS, TUV, PQ)
        # tuv = (t, w, h); out[l,t,n,h,w,i,j] = GT[l,n,(t,w,h),q=(i,j)]
        GT = GT.reshape(LEV, N, T, K7, K7, PQ)     # [l,n,t,w,h,q]
        out = np.ascontiguousarray(
            GT.transpose(0, 2, 1, 4, 3, 5), dtype=np.float32).reshape(
            LEV, B, T, N, K7, K7, K7, K7)
    else:
        G = np.empty((LEV, NCORES, NS, PQ, TUV), np.float32)
        for kc, r in enumerate(res.results):
            G[:, kc] = r["gout"]
        G = G.reshape(LEV, N, PQ, T, K7, K7)       # [l,n,q,t,w,h]
        out = np.ascontiguousarray(
            G.transpose(0, 3, 1, 5, 4, 2), dtype=np.float32).reshape(
            LEV, B, T, N, K7, K7, K7, K7)
    LAST_RESULT['host_post_s'] = _time.time() - _t2
    return out


# revision 7
# speedup vs baseline: 1.0781x; 1.0781x over previous
"""Trainium2 kernel for the DepthTracker correlation pyramid.

Math: for each level l, frame t, track n, the reference bilinearly samples a
7x7 grid of points around coords[t,n] from fmaps_l (128 channels) and
correlates each sample with the 49 track features -> out (L,B,T,N,7,7,7,7).

Decomposition (host gathers + blends patches, device does the 10-GFLOP
correlation, 32 tracks per core, fully data parallel, no collectives):
  out[l,t,n,h,w,pq] = G[l,n,pq,(t,w,h)]
  G[l,n,pq,(t,w,h)] = sum_c trackT[c,(l,n,pq)] * feat[l,n,c,(t,w,h)]

Device layout (2-byte dtypes): the matmul computes G^T per track in
128-row chunks of the TUV=784 axis: out[tuv_chunk(128), pq(49)] =
patch_chunk[C=128,128]^T @ trackT[C,49]. Stationary weights are always
exactly 128 columns so the compiler's Fast Weight Load kicks in; chunks
step by 112 and rows 112:127 of each PSUM result are discarded (they
were computed from the next chunk's columns), so the SBUF/ HBM output
tile is a dense [112, NB*7*49] block with zero padding waste. Stores are
112-partition, 5488B-contiguous-per-partition DMAs: they avoid SDMA
engine 15 (which HW-measures ~17% slower than the others and is the
critical path of a 128-row-store layout) and carry no dead rows (the old
pq-on-partitions layout stored 128 rows with only 98 useful).

HBM traffic per core: 25.8 MB patch loads + 1.6 MB track + 9.8 MB stores
= 37.2 MB vs 40.1 MB for the old layout.

COMPUTE_DT='f32r' + OUT_DT='f32' is a slower, more precise fallback that
uses the old pq-on-partitions layout.
"""

import numpy as np

R = 3
K7 = 7
LEV = 4
B, T, C, N = 1, 16, 128, 256
H, W = 96, 128
NCORES = 8
NS = N // NCORES          # 32 tracks per core
UV = K7 * K7
TUV = T * UV              # 784
PQ = K7 * K7              # 49
CH = (512, 272)           # legacy path matmul free-dim chunks
NB = 8                    # tracks per load DMA / per store
CHUNK = 112               # G^T rows kept per matmul (7*112 = 784)
NCH = TUV // CHUNK        # 7 chunks per track
PAD = 64                  # SBUF-tile-only pad for the weight-read overrun
OPQ = 2752                # gout row f16 elems: NB*NCH*PQ=2744 padded to
#                           5504B so HBM store rows stay 128B-aligned

COMPUTE_DT = 'f16'        # 'f32r' | 'f32' | 'f16' | 'bf16'
OUT_DT = 'f16'            # dtype of the device G output: 'f32' | 'f16'
TRACE = False             # set True to capture an NTFF profile (test.py only)
LAST_RESULT = {}          # phase timings + profile info for test.py

_BASS_CACHE = {}


def _np_compute_dtype():
    if COMPUTE_DT in ('f32r', 'f32'):
        return np.float32
    if COMPUTE_DT == 'f16':
        return np.float16
    import ml_dtypes
    return np.dtype(ml_dtypes.bfloat16)


def _build_bass():
    key = (COMPUTE_DT, OUT_DT)
    if key in _BASS_CACHE:
        return _BASS_CACHE[key]
    import concourse.bacc as bacc
    import concourse.mybir as mybir
    from concourse import tile

    cdt = {
        'f32r': mybir.dt.float32r,
        'f32': mybir.dt.float32,
        'f16': mybir.dt.float16,
        'bf16': mybir.dt.bfloat16,
    }[COMPUTE_DT]
    f32 = mybir.dt.float32
    odt = f32 if OUT_DT == 'f32' else mybir.dt.float16

    nc = bacc.Bacc("TRN2", target_bir_lowering=False, debug=False)
    gt = mybir.dt.size(cdt) == 2  # G^T dense-store layout
    # patches: c-major with each partition's data contiguous per level.
    # Unpadded: load runs stay 12544B / 64B-aligned (a +16-col pad was
    # measured to drop load DMA from 24.8 to 18.0 GB/s/engine). The SBUF
    # tile instead carries a PAD-col junk tail for the final 128-col
    # weight read of each batch (rows computed from it are discarded).
    patches = nc.dram_tensor("patches", (LEV, C, NS * TUV), cdt,
                             kind="ExternalInput")
    trackT = nc.dram_tensor("trackT", (C, LEV * NS * PQ), cdt,
                            kind="ExternalInput")
    if gt:
        gout = nc.dram_tensor("gout", (LEV, NS // NB, CHUNK, OPQ),
                              odt, kind="ExternalOutput")
    else:
        gout = nc.dram_tensor("gout", (LEV, NS, PQ, TUV), odt,
                              kind="ExternalOutput")
    with tile.TileContext(nc) as tc:
        with (
            tc.tile_pool(name="track", bufs=1) as track_pool,
            tc.tile_pool(name="patch",
                         bufs=8 if gt else 3) as patch_pool,
            tc.tile_pool(name="out",
                         bufs=3 if mybir.dt.size(odt) == 2 else 2
                         ) as out_pool,
            tc.tile_pool(name="psum", bufs=8 if gt else 4,
                         space="PSUM") as psum_pool,
        ):
            tr = track_pool.tile([C, LEV * NS * PQ], cdt)
            for l in range(LEV):
                # per-level just-in-time track slice: keeps the first patch
                # load from queuing behind a full track preload
                ksl = l * NS * PQ
                nc.sync.dma_start(tr[:, ksl:ksl + NS * PQ],
                                  trackT[:, ksl:ksl + NS * PQ])
                for nb in range(NS // NB):
                    ext = PAD if gt else 0
                    pt = patch_pool.tile([C, NB * TUV + ext], cdt, tag="pt")
                    off = nb * NB * TUV
                    if l == 0 and nb == 0:
                        # split the first load so compute starts earlier
                        nc.sync.dma_start(
                            pt[:, :NB * TUV // 2],
                            patches[l, :, off:off + NB * TUV // 2])
                        nc.sync.dma_start(
                            pt[:, NB * TUV // 2:NB * TUV],
                            patches[l, :, off + NB * TUV // 2:
                                          off + NB * TUV])
                    else:
                        nc.sync.dma_start(
                            pt[:, :NB * TUV],
                            patches[l, :, off:off + NB * TUV])
                    if gt:
                        ot = out_pool.tile([CHUNK, OPQ], odt, tag="ot")
                        for g in range(NB):
                            n = nb * NB + g
                            k = (l * NS + n) * PQ
                            ps = psum_pool.tile([128, 512], f32, tag="ps")
                            for j in range(NCH):
                                wofs = g * TUV + j * CHUNK
                                nc.tensor.matmul(
                                    ps[:, j * PQ:(j + 1) * PQ],
                                    pt[:, wofs:wofs + 128],
                                    tr[:, k:k + PQ],
                                    start=True, stop=True)
                            dst = ot[:, g * NCH * PQ:(g + 1) * NCH * PQ]
                            if g % 2 == 0:
                                nc.vector.tensor_copy(
                                    dst, ps[0:CHUNK, :NCH * PQ])
                            else:
                                nc.scalar.copy(dst, ps[0:CHUNK, :NCH * PQ])
                        nc.sync.dma_start(gout[l, nb], ot[:])
                    else:
                        ot = out_pool.tile([PQ, NB * TUV], odt, tag="ot")
                        for g in range(NB):
                            n = nb * NB + g
                            k = (l * NS + n) * PQ
                            ps = psum_pool.tile([128, TUV], f32, tag="ps")
                            o = 0
                            for w_ch in CH:
                                nc.tensor.matmul(
                                    ps[0:PQ, o:o + w_ch],
                                    tr[:, k:k + PQ],
                                    pt[:, g * TUV + o:g * TUV + o + w_ch],
                                    start=True, stop=True)
                                o += w_ch
                            dst = ot[0:PQ, g * TUV:(g + 1) * TUV]
                            if g % 2 == 0:
                                nc.vector.tensor_copy(dst, ps[0:PQ, :])
                            else:
                                nc.scalar.copy(dst, ps[0:PQ, :])
                        nc.sync.dma_start(
                            gout[l, nb * NB:(nb + 1) * NB].rearrange(
                                "g p v -> p g v"),
                            ot[:].rearrange("p (g v) -> p g v", g=NB))
    nc.compile()
    _BASS_CACHE[key] = nc
    return nc


def _blend_mats(xy, dim):
    """xy: (T,N) fp32 coords at this level's scale. Returns (origin (T,N)
    int32, S (T,N,7,8) fp32) with reference clamping semantics folded in."""
    d = np.arange(-R, R + 1, dtype=np.float32)
    q = xy[..., None] + d
    qc = np.clip(q, 0.0, dim - 1.0)
    x0 = np.floor(qc)
    w = (qc - x0).astype(np.float32)
    x0i = x0.astype(np.int32)
    x1i = np.minimum(x0i + 1, dim - 1)
    org = np.clip(np.floor(xy).astype(np.int32) - R, 0, dim - 8)
    v0 = x0i - org[..., None]
    v1 = x1i - org[..., None]
    eye = np.eye(8, dtype=np.float32)
    S = eye[v0] * (1.0 - w)[..., None] + eye[v1] * w[..., None]
    return org, S


def kernel(fmaps0, fmaps1, fmaps2, fmaps3, track0, track1, track2, track3,
           coords):
    import time as _time
    _t0 = _time.time()
    fmaps = [fmaps0, fmaps1, fmaps2, fmaps3]
    tracks = [track0, track1, track2, track3]
    cdt_np = _np_compute_dtype()
    gt = cdt_np().itemsize == 2
    coords2 = np.asarray(coords, np.float32)[0]        # (T,N,2)

    # ---- host: blend matrices + patch gather --------------------------------
    patches_all = np.empty((LEV, C, N, T, K7, K7), cdt_np)
    for l in range(LEV):
        Hl, Wl = H >> l, W >> l
        sc = np.float32(2.0 ** l)
        x = (coords2[..., 0] / sc).astype(np.float32)
        y = (coords2[..., 1] / sc).astype(np.float32)
        cx, Sx = _blend_mats(x, Wl)
        cy, Sy = _blend_mats(y, Hl)
        fm = np.asarray(fmaps[l], np.float32)[0]       # (T,C,Hl,Wl)
        iy = cy[..., None] + np.arange(8)              # (T,N,8)
        ix = cx[..., None] + np.arange(8)
        t_idx = np.arange(T)[:, None, None, None]
        # fancy indexing -> (T,N,8,8,C) over (u=y-row, v=x-col)
        p = fm[t_idx, :, iy[:, :, :, None], ix[:, :, None, :]]
        # x-blend: (T,N,1,7,8) @ (T,N,8,8,C) -> (T,N,8,7,C)  [u, h]
        px = np.matmul(Sx[:, :, None, :, :], p)
        # y-blend: (T,N,7,8) @ (T,N,8,7*C) -> (T,N,7,7,C)    [w, h]
        py = np.matmul(Sy, px.reshape(T, N, 8, K7 * C))
        py = py.reshape(T, N, K7, K7, C)
        patches_all[l] = py.transpose(4, 1, 0, 2, 3)   # (C,N,T,7,7)

    trackT_all = np.empty((C, LEV, N, PQ), cdt_np)
    for l in range(LEV):
        # track_l: (1,49,N,C) -> (C, N, PQ)
        trackT_all[:, l] = np.asarray(tracks[l], np.float32)[0].transpose(2, 1, 0)

    # ---- device: G = track^T @ patches, 32 tracks per core ------------------
    nc = _build_bass()
    from concourse import bass_utils
    in_maps = []
    for kc in range(NCORES):
        sl = slice(kc * NS, (kc + 1) * NS)
        in_maps.append({
            "patches": np.ascontiguousarray(
                patches_all[:, :, sl].reshape(LEV, C, NS * TUV)),
            "trackT": np.ascontiguousarray(
                trackT_all[:, :, sl].reshape(C, LEV * NS * PQ)),
        })
    _t1 = _time.time()
    res = bass_utils.run_bass_kernel_spmd(
        nc, in_maps, core_ids=list(range(NCORES)), trace=TRACE)
    _t2 = _time.time()
    LAST_RESULT.update(
        host_pre_s=_t1 - _t0, spmd_s=_t2 - _t1,
        exec_time_ns=res.exec_time_ns, profile_json=res.profile_json)
    if gt:
        # per core: gout (LEV, NS//NB, 112, OPQ): row p of chunk j of
        # track g is G^T[l, n=nb*NB+g, tuv=j*112+p, pq]; cols 2744: junk.
        GT = np.empty((LEV, NCORES, NS, TUV, PQ), np.float32)
        for kc, r in enumerate(res.results):
            g = r["gout"][..., :NB * NCH * PQ].reshape(
                LEV, NS // NB, CHUNK, NB, NCH, PQ)
            GT[:, kc] = g.transpose(0, 1, 3, 4, 2, 5).reshape(
                LEV, NS, TUV, PQ)
        # tuv = (t, w, h); out[l,t,n,h,w,i,j] = GT[l,n,(t,w,h),q=(i,j)]
        GT = GT.reshape(LEV, N, T, K7, K7, PQ)     # [l,n,t,w,h,q]
        out = np.ascontiguousarray(
            GT.transpose(0, 2, 1, 4, 3, 5), dtype=np.float32).reshape(
            LEV, B, T, N, K7, K7, K7, K7)
    else:
        G = np.empty((LEV, NCORES, NS, PQ, TUV), np.float32)
        for kc, r in enumerate(res.results):
            G[:, kc] = r["gout"]
        G = G.reshape(LEV, N, PQ, T, K7, K7)       # [l,n,q,t,w,h]
        out = np.ascontiguousarray(
            G.transpose(0, 3, 1, 5, 4, 2), dtype=np.float32).reshape(
            LEV, B, T, N, K7, K7, K7, K7)
    LAST_RESULT['host_post_s'] = _time.time() - _t2
    return out


# revision 12
# speedup vs baseline: 1.2280x; 1.1391x over previous
"""Trainium2 kernel for the DepthTracker correlation pyramid.

Math: for each level l, frame t, track n, the reference bilinearly samples a
7x7 grid of points around coords[t,n] from fmaps_l (128 channels) and
correlates each sample with the 49 track features -> out (L,B,T,N,7,7,7,7).

Decomposition (host gathers + blends patches, device does the 10-GFLOP
correlation, 32 tracks per core, fully data parallel, no collectives):
  out[l,t,n,h,w,pq] = G[l,n,pq,(t,w,h)]
  G[l,n,pq,(t,w,h)] = sum_c trackT[c,(l,n,pq)] * feat[l,n,c,(t,w,h)]

Device layout (2-byte dtypes): the matmul computes G^T per track in
128-row chunks of the TUV=784 axis: out[tuv_chunk(128), pq(49)] =
patch_chunk[C=128,128]^T @ trackT[C,49]. Stationary weights are always
exactly 128 columns so the compiler's Fast Weight Load kicks in; chunks
step by 112 and rows 112:127 of each PSUM result are discarded (they
were computed from the next chunk's columns), so the SBUF/ HBM output
tile is a dense [112, NB*7*49] block with zero padding waste. Stores are
112-partition, 5488B-contiguous-per-partition DMAs: they avoid SDMA
engine 15 (which HW-measures ~17% slower than the others and is the
critical path of a 128-row-store layout) and carry no dead rows (the old
pq-on-partitions layout stored 128 rows with only 98 useful).

HBM traffic per core: 25.8 MB patch loads + 1.6 MB track + 9.8 MB stores
= 37.2 MB vs 40.1 MB for the old layout.

COMPUTE_DT='f32r' + OUT_DT='f32' is a slower, more precise fallback that
uses the old pq-on-partitions layout.
"""

import numpy as np

R = 3
K7 = 7
LEV = 4
B, T, C, N = 1, 16, 128, 256
H, W = 96, 128
NCORES = 8
NS = N // NCORES          # 32 tracks per core
UV = K7 * K7
TUV = T * UV              # 784
PQ = K7 * K7              # 49
CH = (512, 272)           # legacy path matmul free-dim chunks
NB = 8                    # tracks per load DMA / per store
CHUNK = 112               # G^T rows kept per matmul (7*112 = 784)
NCH = TUV // CHUNK        # 7 chunks per track
PAD = 64                  # SBUF-tile-only pad for the weight-read overrun
OPQ = 2752                # gout row f16 elems: NB*NCH*PQ=2744 padded to
#                           5504B so HBM store rows stay 128B-aligned

COMPUTE_DT = 'f16'        # 'f32r' | 'f32' | 'f16' | 'bf16'
OUT_DT = 'f16'            # dtype of the device G output: 'f32' | 'f16'
TRACE = False             # set True to capture an NTFF profile (test.py only)
LAST_RESULT = {}          # phase timings + profile info for test.py

_BASS_CACHE = {}


def _np_compute_dtype():
    if COMPUTE_DT in ('f32r', 'f32'):
        return np.float32
    if COMPUTE_DT == 'f16':
        return np.float16
    import ml_dtypes
    return np.dtype(ml_dtypes.bfloat16)


def _build_bass():
    key = (COMPUTE_DT, OUT_DT)
    if key in _BASS_CACHE:
        return _BASS_CACHE[key]
    import concourse.bacc as bacc
    import concourse.mybir as mybir
    from concourse import tile

    cdt = {
        'f32r': mybir.dt.float32r,
        'f32': mybir.dt.float32,
        'f16': mybir.dt.float16,
        'bf16': mybir.dt.bfloat16,
    }[COMPUTE_DT]
    f32 = mybir.dt.float32
    odt = f32 if OUT_DT == 'f32' else mybir.dt.float16

    nc = bacc.Bacc("TRN2", target_bir_lowering=False, debug=False)
    gt = mybir.dt.size(cdt) == 2  # G^T dense-store layout
    # patches: c-major with each partition's data contiguous per level.
    # Unpadded: load runs stay 12544B / 64B-aligned (a +16-col pad was
    # measured to drop load DMA from 24.8 to 18.0 GB/s/engine). The SBUF
    # tile instead carries a PAD-col junk tail for the final 128-col
    # weight read of each batch (rows computed from it are discarded).
    patches = nc.dram_tensor("patches", (LEV, C, NS * TUV), cdt,
                             kind="ExternalInput")
    trackT = nc.dram_tensor("trackT", (C, LEV * NS * PQ), cdt,
                            kind="ExternalInput")
    if gt:
        gout = nc.dram_tensor("gout", (LEV, NS // NB, CHUNK, OPQ),
                              odt, kind="ExternalOutput")
    else:
        gout = nc.dram_tensor("gout", (LEV, NS, PQ, TUV), odt,
                              kind="ExternalOutput")
    with tile.TileContext(nc) as tc:
        with (
            tc.tile_pool(name="track", bufs=1) as track_pool,
            tc.tile_pool(name="patch",
                         bufs=8 if gt else 3) as patch_pool,
            tc.tile_pool(name="out",
                         bufs=4 if mybir.dt.size(odt) == 2 else 2
                         ) as out_pool,
            tc.tile_pool(name="psum", bufs=8 if gt else 4,
                         space="PSUM") as psum_pool,
        ):
            tr = track_pool.tile([C, LEV * NS * PQ], cdt)
            # Stores are issued D batches late in program order. All DMAs
            # share the sync HWDGE ring (mixing in the scalar ring corrupts
            # the shared DMAHW0-7 completion-count lanes when the rings
            # finish out of order); a store issue stalled on its copies
            # would block every later patch-load issue behind it, so by the
            # time a store issues, its copies are long done.
            D = 2
            pending = []
            for l in range(LEV):
                # per-level just-in-time track slice: keeps the first patch
                # load from queuing behind a full track preload
                ksl = l * NS * PQ
                nc.sync.dma_start(tr[:, ksl:ksl + NS * PQ],
                                  trackT[:, ksl:ksl + NS * PQ])
                for nb in range(NS // NB):
                    ext = PAD if gt else 0
                    pt = patch_pool.tile([C, NB * TUV + ext], cdt, tag="pt")
                    off = nb * NB * TUV
                    if l == 0 and nb == 0:
                        # split the first load so compute starts earlier
                        nc.sync.dma_start(
                            pt[:, :NB * TUV // 2],
                            patches[l, :, off:off + NB * TUV // 2])
                        nc.sync.dma_start(
                            pt[:, NB * TUV // 2:NB * TUV],
                            patches[l, :, off + NB * TUV // 2:
                                          off + NB * TUV])
                    else:
                        nc.sync.dma_start(
                            pt[:, :NB * TUV],
                            patches[l, :, off:off + NB * TUV])
                    if gt:
                        ot = out_pool.tile([CHUNK, OPQ], odt, tag="ot")
                        for g in range(NB):
                            n = nb * NB + g
                            k = (l * NS + n) * PQ
                            ps = psum_pool.tile([128, 512], f32, tag="ps")
                            for j in range(NCH):
                                wofs = g * TUV + j * CHUNK
                                nc.tensor.matmul(
                                    ps[:, j * PQ:(j + 1) * PQ],
                                    pt[:, wofs:wofs + 128],
                                    tr[:, k:k + PQ],
                                    start=True, stop=True)
                            dst = ot[:, g * NCH * PQ:(g + 1) * NCH * PQ]
                            if g % 2 == 0:
                                nc.vector.tensor_copy(
                                    dst, ps[0:CHUNK, :NCH * PQ])
                            else:
                                nc.scalar.copy(dst, ps[0:CHUNK, :NCH * PQ])
                        pending.append((gout[l, nb], ot))
                        if len(pending) > D:
                            dst, src = pending.pop(0)
                            nc.sync.dma_start(dst, src[:])
                    else:
                        ot = out_pool.tile([PQ, NB * TUV], odt, tag="ot")
                        for g in range(NB):
                            n = nb * NB + g
                            k = (l * NS + n) * PQ
                            ps = psum_pool.tile([128, TUV], f32, tag="ps")
                            o = 0
                            for w_ch in CH:
                                nc.tensor.matmul(
                                    ps[0:PQ, o:o + w_ch],
                                    tr[:, k:k + PQ],
                                    pt[:, g * TUV + o:g * TUV + o + w_ch],
                                    start=True, stop=True)
                                o += w_ch
                            dst = ot[0:PQ, g * TUV:(g + 1) * TUV]
                            if g % 2 == 0:
                                nc.vector.tensor_copy(dst, ps[0:PQ, :])
                            else:
                                nc.scalar.copy(dst, ps[0:PQ, :])
                        nc.sync.dma_start(
                            gout[l, nb * NB:(nb + 1) * NB].rearrange(
                                "g p v -> p g v"),
                            ot[:].rearrange("p (g v) -> p g v", g=NB))
            for dst, src in pending:
                nc.sync.dma_start(dst, src[:])
    nc.compile()
    _BASS_CACHE[key] = nc
    return nc


def _blend_mats(xy, dim):
    """xy: (T,N) fp32 coords at this level's scale. Returns (origin (T,N)
    int32, S (T,N,7,8) fp32) with reference clamping semantics folded in."""
    d = np.arange(-R, R + 1, dtype=np.float32)
    q = xy[..., None] + d
    qc = np.clip(q, 0.0, dim - 1.0)
    x0 = np.floor(qc)
    w = (qc - x0).astype(np.float32)
    x0i = x0.astype(np.int32)
    x1i = np.minimum(x0i + 1, dim - 1)
    org = np.clip(np.floor(xy).astype(np.int32) - R, 0, dim - 8)
    v0 = x0i - org[..., None]
    v1 = x1i - org[..., None]
    eye = np.eye(8, dtype=np.float32)
    S = eye[v0] * (1.0 - w)[..., None] + eye[v1] * w[..., None]
    return org, S


def kernel(fmaps0, fmaps1, fmaps2, fmaps3, track0, track1, track2, track3,
           coords):
    import time as _time
    _t0 = _time.time()
    fmaps = [fmaps0, fmaps1, fmaps2, fmaps3]
    tracks = [track0, track1, track2, track3]
    cdt_np = _np_compute_dtype()
    gt = cdt_np().itemsize == 2
    coords2 = np.asarray(coords, np.float32)[0]        # (T,N,2)

    # ---- host: blend matrices + patch gather --------------------------------
    patches_all = np.empty((LEV, C, N, T, K7, K7), cdt_np)
    for l in range(LEV):
        Hl, Wl = H >> l, W >> l
        sc = np.float32(2.0 ** l)
        x = (coords2[..., 0] / sc).astype(np.float32)
        y = (coords2[..., 1] / sc).astype(np.float32)
        cx, Sx = _blend_mats(x, Wl)
        cy, Sy = _blend_mats(y, Hl)
        fm = np.asarray(fmaps[l], np.float32)[0]       # (T,C,Hl,Wl)
        iy = cy[..., None] + np.arange(8)              # (T,N,8)
        ix = cx[..., None] + np.arange(8)
        t_idx = np.arange(T)[:, None, None, None]
        # fancy indexing -> (T,N,8,8,C) over (u=y-row, v=x-col)
        p = fm[t_idx, :, iy[:, :, :, None], ix[:, :, None, :]]
        # x-blend: (T,N,1,7,8) @ (T,N,8,8,C) -> (T,N,8,7,C)  [u, h]
        px = np.matmul(Sx[:, :, None, :, :], p)
        # y-blend: (T,N,7,8) @ (T,N,8,7*C) -> (T,N,7,7,C)    [w, h]
        py = np.matmul(Sy, px.reshape(T, N, 8, K7 * C))
        py = py.reshape(T, N, K7, K7, C)
        patches_all[l] = py.transpose(4, 1, 0, 2, 3)   # (C,N,T,7,7)

    trackT_all = np.empty((C, LEV, N, PQ), cdt_np)
    for l in range(LEV):
        # track_l: (1,49,N,C) -> (C, N, PQ)
        trackT_all[:, l] = np.asarray(tracks[l], np.float32)[0].transpose(2, 1, 0)

    # ---- device: G = track^T @ patches, 32 tracks per core ------------------
    nc = _build_bass()
    from concourse import bass_utils
    in_maps = []
    for kc in range(NCORES):
        sl = slice(kc * NS, (kc + 1) * NS)
        in_maps.append({
            "patches": np.ascontiguousarray(
                patches_all[:, :, sl].reshape(LEV, C, NS * TUV)),
            "trackT": np.ascontiguousarray(
                trackT_all[:, :, sl].reshape(C, LEV * NS * PQ)),
        })
    _t1 = _time.time()
    res = bass_utils.run_bass_kernel_spmd(
        nc, in_maps, core_ids=list(range(NCORES)), trace=TRACE)
    _t2 = _time.time()
    LAST_RESULT.update(
        host_pre_s=_t1 - _t0, spmd_s=_t2 - _t1,
        exec_time_ns=res.exec_time_ns, profile_json=res.profile_json)
    if gt:
        # per core: gout (LEV, NS//NB, 112, OPQ): row p of chunk j of
        # track g is G^T[l, n=nb*NB+g, tuv=j*112+p, pq]; cols 2744: junk.
        GT = np.empty((LEV, NCORES, NS, TUV, PQ), np.float32)
        for kc, r in enumerate(res.results):
            g = r["gout"][..., :NB * NCH * PQ].reshape(
                LEV, NS // NB, CHUNK, NB, NCH, PQ)
            GT[:, kc] = g.transpose(0, 1, 3, 4, 2, 5).reshape(
                LEV, NS, TUV, PQ)
        # tuv = (t, w, h); out[l,t,n,h,w,i,j] = GT[l,n,(t,w,h),q=(i,j)]
        GT = GT.reshape(LEV, N, T, K7, K7, PQ)     # [l,n,t,w,h,q]
        out = np.ascontiguousarray(
            GT.transpose(0, 2, 1, 4, 3, 5), dtype=np.float32).reshape(
            LEV, B, T, N, K7, K7, K7, K7)
    else:
        G = np.empty((LEV, NCORES, NS, PQ, TUV), np.float32)
        for kc, r in enumerate(res.results):
            G[:, kc] = r["gout"]
        G = G.reshape(LEV, N, PQ, T, K7, K7)       # [l,n,q,t,w,h]
        out = np.ascontiguousarray(
            G.transpose(0, 3, 1, 5, 4, 2), dtype=np.float32).reshape(
            LEV, B, T, N, K7, K7, K7, K7)
    LAST_RESULT['host_post_s'] = _time.time() - _t2
    return out


# revision 14
# speedup vs baseline: 1.2452x; 1.0140x over previous
"""Trainium2 kernel for the DepthTracker correlation pyramid.

Math: for each level l, frame t, track n, the reference bilinearly samples a
7x7 grid of points around coords[t,n] from fmaps_l (128 channels) and
correlates each sample with the 49 track features -> out (L,B,T,N,7,7,7,7).

Decomposition (host gathers + blends patches, device does the 10-GFLOP
correlation, 32 tracks per core, fully data parallel, no collectives):
  out[l,t,n,h,w,pq] = G[l,n,pq,(t,w,h)]
  G[l,n,pq,(t,w,h)] = sum_c trackT[c,(l,n,pq)] * feat[l,n,c,(t,w,h)]

Device layout (2-byte dtypes): the matmul computes G^T per track in
128-row chunks of the TUV=784 axis: out[tuv_chunk(128), pq(49)] =
patch_chunk[C=128,128]^T @ trackT[C,49]. Stationary weights are always
exactly 128 columns so the compiler's Fast Weight Load kicks in; chunks
step by 112 and rows 112:127 of each PSUM result are discarded (they
were computed from the next chunk's columns), so the SBUF/ HBM output
tile is a dense [112, NB*7*49] block with zero padding waste. Stores are
112-partition, 5488B-contiguous-per-partition DMAs: they avoid SDMA
engine 15 (which HW-measures ~17% slower than the others and is the
critical path of a 128-row-store layout) and carry no dead rows (the old
pq-on-partitions layout stored 128 rows with only 98 useful).

HBM traffic per core: 25.8 MB patch loads + 1.6 MB track + 9.8 MB stores
= 37.2 MB vs 40.1 MB for the old layout.

COMPUTE_DT='f32r' + OUT_DT='f32' is a slower, more precise fallback that
uses the old pq-on-partitions layout.
"""

import numpy as np

R = 3
K7 = 7
LEV = 4
B, T, C, N = 1, 16, 128, 256
H, W = 96, 128
NCORES = 8
NS = N // NCORES          # 32 tracks per core
UV = K7 * K7
TUV = T * UV              # 784
PQ = K7 * K7              # 49
CH = (512, 272)           # legacy path matmul free-dim chunks
NB = 8                    # tracks per load DMA / per store
CHUNK = 112               # G^T rows kept per matmul (7*112 = 784)
NCH = TUV // CHUNK        # 7 chunks per track
PAD = 64                  # SBUF-tile-only pad for the weight-read overrun
OPQ = 2752                # gout row f16 elems: NB*NCH*PQ=2744 padded to
#                           5504B so HBM store rows stay 128B-aligned

COMPUTE_DT = 'f16'        # 'f32r' | 'f32' | 'f16' | 'bf16'
OUT_DT = 'f16'            # dtype of the device G output: 'f32' | 'f16'
TRACE = False             # set True to capture an NTFF profile (test.py only)
LAST_RESULT = {}          # phase timings + profile info for test.py

_BASS_CACHE = {}


def _np_compute_dtype():
    if COMPUTE_DT in ('f32r', 'f32'):
        return np.float32
    if COMPUTE_DT == 'f16':
        return np.float16
    import ml_dtypes
    return np.dtype(ml_dtypes.bfloat16)


def _build_bass():
    key = (COMPUTE_DT, OUT_DT)
    if key in _BASS_CACHE:
        return _BASS_CACHE[key]
    import concourse.bacc as bacc
    import concourse.mybir as mybir
    from concourse import tile

    cdt = {
        'f32r': mybir.dt.float32r,
        'f32': mybir.dt.float32,
        'f16': mybir.dt.float16,
        'bf16': mybir.dt.bfloat16,
    }[COMPUTE_DT]
    f32 = mybir.dt.float32
    odt = f32 if OUT_DT == 'f32' else mybir.dt.float16

    nc = bacc.Bacc("TRN2", target_bir_lowering=False, debug=False)
    gt = mybir.dt.size(cdt) == 2  # G^T dense-store layout
    # patches: c-major with each partition's data contiguous per level.
    # Unpadded: load runs stay 12544B / 64B-aligned (a +16-col pad was
    # measured to drop load DMA from 24.8 to 18.0 GB/s/engine). The SBUF
    # tile instead carries a PAD-col junk tail for the final 128-col
    # weight read of each batch (rows computed from it are discarded).
    patches = nc.dram_tensor("patches", (LEV, C, NS * TUV), cdt,
                             kind="ExternalInput")
    trackT = nc.dram_tensor("trackT", (C, LEV * NS * PQ), cdt,
                            kind="ExternalInput")
    if gt:
        gout = nc.dram_tensor("gout", (LEV, NS // NB, CHUNK, OPQ),
                              odt, kind="ExternalOutput")
    else:
        gout = nc.dram_tensor("gout", (LEV, NS, PQ, TUV), odt,
                              kind="ExternalOutput")
    with tile.TileContext(nc) as tc:
        with (
            tc.tile_pool(name="track", bufs=1) as track_pool,
            tc.tile_pool(name="patch",
                         bufs=8 if gt else 3) as patch_pool,
            tc.tile_pool(name="out",
                         bufs=4 if mybir.dt.size(odt) == 2 else 2
                         ) as out_pool,
            tc.tile_pool(name="psum", bufs=8 if gt else 4,
                         space="PSUM") as psum_pool,
        ):
            tr = track_pool.tile([C, LEV * NS * PQ], cdt)
            # Stores are issued D batches late in program order. All DMAs
            # share the sync HWDGE ring (mixing in the scalar ring corrupts
            # the shared DMAHW0-7 completion-count lanes when the rings
            # finish out of order); a store issue stalled on its copies
            # would block every later patch-load issue behind it, so by the
            # time a store issues, its copies are long done.
            D = 2
            pending = []
            # all track slices load upfront (4 DMAs so the first matmul
            # only waits on slice 0). Loading per level looked cheaper but
            # re-writing the tr tile made each level's track load wait on
            # every matmul of the previous level (WAR), stalling the sync
            # ring and starving the patch-load queue at level boundaries.
            for l in range(LEV):
                ksl = l * NS * PQ
                nc.sync.dma_start(tr[:, ksl:ksl + NS * PQ],
                                  trackT[:, ksl:ksl + NS * PQ])
            for l in range(LEV):
                for nb in range(NS // NB):
                    ext = PAD if gt else 0
                    pt = patch_pool.tile([C, NB * TUV + ext], cdt, tag="pt")
                    off = nb * NB * TUV
                    if l == 0 and nb == 0:
                        # split the first load so compute starts earlier
                        nc.sync.dma_start(
                            pt[:, :NB * TUV // 2],
                            patches[l, :, off:off + NB * TUV // 2])
                        nc.sync.dma_start(
                            pt[:, NB * TUV // 2:NB * TUV],
                            patches[l, :, off + NB * TUV // 2:
                                          off + NB * TUV])
                    else:
                        nc.sync.dma_start(
                            pt[:, :NB * TUV],
                            patches[l, :, off:off + NB * TUV])
                    if gt:
                        ot = out_pool.tile([CHUNK, OPQ], odt, tag="ot")
                        for g in range(NB):
                            n = nb * NB + g
                            k = (l * NS + n) * PQ
                            ps = psum_pool.tile([128, 512], f32, tag="ps")
                            for j in range(NCH):
                                wofs = g * TUV + j * CHUNK
                                nc.tensor.matmul(
                                    ps[:, j * PQ:(j + 1) * PQ],
                                    pt[:, wofs:wofs + 128],
                                    tr[:, k:k + PQ],
                                    start=True, stop=True)
                            dst = ot[:, g * NCH * PQ:(g + 1) * NCH * PQ]
                            if g % 2 == 0:
                                nc.vector.tensor_copy(
                                    dst, ps[0:CHUNK, :NCH * PQ])
                            else:
                                nc.scalar.copy(dst, ps[0:CHUNK, :NCH * PQ])
                        pending.append((gout[l, nb], ot))
                        if len(pending) > D:
                            dst, src = pending.pop(0)
                            nc.sync.dma_start(dst, src[:])
                    else:
                        ot = out_pool.tile([PQ, NB * TUV], odt, tag="ot")
                        for g in range(NB):
                            n = nb * NB + g
                            k = (l * NS + n) * PQ
                            ps = psum_pool.tile([128, TUV], f32, tag="ps")
                            o = 0
                            for w_ch in CH:
                                nc.tensor.matmul(
                                    ps[0:PQ, o:o + w_ch],
                                    tr[:, k:k + PQ],
                                    pt[:, g * TUV + o:g * TUV + o + w_ch],
                                    start=True, stop=True)
                                o += w_ch
                            dst = ot[0:PQ, g * TUV:(g + 1) * TUV]
                            if g % 2 == 0:
                                nc.vector.tensor_copy(dst, ps[0:PQ, :])
                            else:
                                nc.scalar.copy(dst, ps[0:PQ, :])
                        nc.sync.dma_start(
                            gout[l, nb * NB:(nb + 1) * NB].rearrange(
                                "g p v -> p g v"),
                            ot[:].rearrange("p (g v) -> p g v", g=NB))
            for i, (dst, src) in enumerate(pending):
                if i == len(pending) - 1:
                    # split the last store so its first half can drain
                    # while the final tracks' copies finish
                    nc.sync.dma_start(dst[:, :OPQ // 2], src[:, :OPQ // 2])
                    nc.sync.dma_start(dst[:, OPQ // 2:], src[:, OPQ // 2:])
                else:
                    nc.sync.dma_start(dst, src[:])
    nc.compile()
    _BASS_CACHE[key] = nc
    return nc


def _blend_mats(xy, dim):
    """xy: (T,N) fp32 coords at this level's scale. Returns (origin (T,N)
    int32, S (T,N,7,8) fp32) with reference clamping semantics folded in."""
    d = np.arange(-R, R + 1, dtype=np.float32)
    q = xy[..., None] + d
    qc = np.clip(q, 0.0, dim - 1.0)
    x0 = np.floor(qc)
    w = (qc - x0).astype(np.float32)
    x0i = x0.astype(np.int32)
    x1i = np.minimum(x0i + 1, dim - 1)
    org = np.clip(np.floor(xy).astype(np.int32) - R, 0, dim - 8)
    v0 = x0i - org[..., None]
    v1 = x1i - org[..., None]
    eye = np.eye(8, dtype=np.float32)
    S = eye[v0] * (1.0 - w)[..., None] + eye[v1] * w[..., None]
    return org, S


def kernel(fmaps0, fmaps1, fmaps2, fmaps3, track0, track1, track2, track3,
           coords):
    import time as _time
    _t0 = _time.time()
    fmaps = [fmaps0, fmaps1, fmaps2, fmaps3]
    tracks = [track0, track1, track2, track3]
    cdt_np = _np_compute_dtype()
    gt = cdt_np().itemsize == 2
    coords2 = np.asarray(coords, np.float32)[0]        # (T,N,2)

    # ---- host: blend matrices + patch gather --------------------------------
    patches_all = np.empty((LEV, C, N, T, K7, K7), cdt_np)
    for l in range(LEV):
        Hl, Wl = H >> l, W >> l
        sc = np.float32(2.0 ** l)
        x = (coords2[..., 0] / sc).astype(np.float32)
        y = (coords2[..., 1] / sc).astype(np.float32)
        cx, Sx = _blend_mats(x, Wl)
        cy, Sy = _blend_mats(y, Hl)
        fm = np.asarray(fmaps[l], np.float32)[0]       # (T,C,Hl,Wl)
        iy = cy[..., None] + np.arange(8)              # (T,N,8)
        ix = cx[..., None] + np.arange(8)
        t_idx = np.arange(T)[:, None, None, None]
        # fancy indexing -> (T,N,8,8,C) over (u=y-row, v=x-col)
        p = fm[t_idx, :, iy[:, :, :, None], ix[:, :, None, :]]
        # x-blend: (T,N,1,7,8) @ (T,N,8,8,C) -> (T,N,8,7,C)  [u, h]
        px = np.matmul(Sx[:, :, None, :, :], p)
        # y-blend: (T,N,7,8) @ (T,N,8,7*C) -> (T,N,7,7,C)    [w, h]
        py = np.matmul(Sy, px.reshape(T, N, 8, K7 * C))
        py = py.reshape(T, N, K7, K7, C)
        patches_all[l] = py.transpose(4, 1, 0, 2, 3)   # (C,N,T,7,7)

    trackT_all = np.empty((C, LEV, N, PQ), cdt_np)
    for l in range(LEV):
        # track_l: (1,49,N,C) -> (C, N, PQ)
        trackT_all[:, l] = np.asarray(tracks[l], np.float32)[0].transpose(2, 1, 0)

    # ---- device: G = track^T @ patches, 32 tracks per core ------------------
    nc = _build_bass()
    from concourse import bass_utils
    in_maps = []
    for kc in range(NCORES):
        sl = slice(kc * NS, (kc + 1) * NS)
        in_maps.append({
            "patches": np.ascontiguousarray(
                patches_all[:, :, sl].reshape(LEV, C, NS * TUV)),
            "trackT": np.ascontiguousarray(
                trackT_all[:, :, sl].reshape(C, LEV * NS * PQ)),
        })
    _t1 = _time.time()
    res = bass_utils.run_bass_kernel_spmd(
        nc, in_maps, core_ids=list(range(NCORES)), trace=TRACE)
    _t2 = _time.time()
    LAST_RESULT.update(
        host_pre_s=_t1 - _t0, spmd_s=_t2 - _t1,
        exec_time_ns=res.exec_time_ns, profile_json=res.profile_json)
    if gt:
        # per core: gout (LEV, NS//NB, 112, OPQ): row p of chunk j of
        # track g is G^T[l, n=nb*NB+g, tuv=j*112+p, pq]; cols 2744: junk.
        GT = np.empty((LEV, NCORES, NS, TUV, PQ), np.float32)
        for kc, r in enumerate(res.results):
            g = r["gout"][..., :NB * NCH * PQ].reshape(
                LEV, NS // NB, CHUNK, NB, NCH, PQ)
            GT[:, kc] = g.transpose(0, 1, 3, 4, 2, 5).reshape(
                LEV, NS, TUV, PQ)
        # tuv = (t, w, h); out[l,t,n,h,w,i,j] = GT[l,n,(t,w,h),q=(i,j)]
        GT = GT.reshape(LEV, N, T, K7, K7, PQ)     # [l,n,t,w,h,q]
        out = np.ascontiguousarray(
            GT.transpose(0, 2, 1, 4, 3, 5), dtype=np.float32).reshape(
            LEV, B, T, N, K7, K7, K7, K7)
    else:
        G = np.empty((LEV, NCORES, NS, PQ, TUV), np.float32)
        for kc, r in enumerate(res.results):
            G[:, kc] = r["gout"]
        G = G.reshape(LEV, N, PQ, T, K7, K7)       # [l,n,q,t,w,h]
        out = np.ascontiguousarray(
            G.transpose(0, 3, 1, 5, 4, 2), dtype=np.float32).reshape(
            LEV, B, T, N, K7, K7, K7, K7)
    LAST_RESULT['host_post_s'] = _time.time() - _t2
    return out
